# revision 14
# baseline (speedup 1.0000x reference)
"""ConversationAwareSAGEConv on 8 Trainium2 NeuronCores (Bass/Tile).

Algorithm notes:
- The per-edge MLP message e = concat(h,ctx)[src] @ Wm + bm depends only on
  the src node, so LN(e)+relu is computed per NODE (N rows) instead of per
  edge (E rows): 10x less work.
- Nodes are relabeled by a degree-balanced permutation so every 128-node
  destination window receives a near-equal number of edges.
- Sharding: nodes are split into 8 contiguous shards. Each core computes msg
  for its shard (node-parallel matmul + LN). The [h|msg] table is all-gathered
  in NCC chunks, each into its own Shared buffer, pipelining the collective
  behind phase 1. Edges are classified by the src node's table chunk, which
  also keeps gather indices < 13.5k (int16-safe).
- Edge rows are fetched with gpsimd.dma_gather (InstDMAGatherAnt): ONE SWDGE
  instruction gathers up to KMAX rows (row i -> partition i%128, slot i//128),
  amortizing the ~1us per-instruction descriptor-generation cost that
  dominates indirect_dma_start (the hardware caps one instruction at ~1-2k
  descriptors, hence KMAX).
- Segment sums run transposed: matmul(lhsT=rows, rhs=onehot) accumulates
  nmT/caT = (features x dst-slot) directly in PSUM; 1/cnt is baked into the
  one-hot host-side; SAGE/gate biases fold into scalar-engine activations;
  the output is written transposed and the host untransposes.
"""
import sys

sys.path.insert(0, "/opt/trn_rl_repo")

import numpy as np
import ml_dtypes

import concourse.bass as bass
import concourse.bacc as bacc
import concourse.tile as tile
from concourse import mybir
import concourse.bass_utils as bass_utils

BF16 = ml_dtypes.bfloat16
EPS = 1e-5
FP = mybir.dt.float32
BF = mybir.dt.bfloat16
I16 = mybir.dt.int16
KMAX = 1024                                   # rows per dma_gather instruction


class Cfg:
    def __init__(self, N, E, C=8, D=128):
        assert N % C == 0
        self.N, self.E, self.C, self.D = N, E, C, D
        self.NS = N // C                      # nodes per shard
        self.W = (self.NS + 127) // 128       # windows per core
        self.NSP = self.W * 128
        self.lastw = self.NS - (self.W - 1) * 128
        self.TW = 256                         # table row: h|msg
        self.G = 7                            # windows per gather group
        self.NCC = 2                          # collective chunks
        q, r = divmod(self.W, self.NCC)
        self.wcnt = [q + (1 if k < r else 0) for k in range(self.NCC)]
        self.wstart = np.concatenate([[0], np.cumsum(self.wcnt)]).astype(int)
        self.rows_cc = []
        for k in range(self.NCC):
            self.rows_cc.append(sum(
                128 if w < self.W - 1 else self.lastw
                for w in range(self.wstart[k], self.wstart[k + 1])))
        # filled by prep_inputs: kc[k][w] = gather chunks for (window, cchunk)
        self.kc = None


# ----------------------------------------------------------------- host prep

def _balanced_perm(dst, cfg):
    """new_id[old] such that every 128-node window gets ~equal in-edges."""
    import heapq
    N, C, W, NS = cfg.N, cfg.C, cfg.W, cfg.NS
    deg = np.bincount(dst, minlength=N)
    order = np.argsort(-deg, kind="stable")
    caps = np.full(C * W, 128, np.int64)
    caps[W - 1::W] = cfg.lastw
    heap = [(0, int(w)) for w in range(C * W)]
    heapq.heapify(heap)
    fill = np.zeros(C * W, np.int64)
    new_id = np.empty(N, np.int64)
    for old in order:
        d = int(deg[old])
        while True:
            load, w = heapq.heappop(heap)
            if fill[w] < caps[w]:
                break
        new_id[old] = (w // W) * NS + (w % W) * 128 + fill[w]
        fill[w] += 1
        if fill[w] < caps[w]:
            heapq.heappush(heap, (load + d, w))
    return new_id


def _wrap_idx(seq):
    """[K] -> [128, K//16] int16 in the swdge wrapped+replicated layout."""
    blk = seq.reshape(-1, 16).T.astype(np.int16)      # [16, K//16]
    return np.tile(blk, (8, 1))


def prep_inputs(h, ctx, src, dst, W_self, W_neigh, b_sage, Wm, bm, ln_g, ln_b,
                Wg, bg, cfg):
    N, C, NS, W, D, G = cfg.N, cfg.C, cfg.NS, cfg.W, cfg.D, cfg.G
    NCC = cfg.NCC
    new_id = _balanced_perm(np.asarray(dst), cfg)
    inv = np.empty(N, np.int64)
    inv[new_id] = np.arange(N)

    X = np.concatenate([h, ctx], axis=1).astype(BF16)   # [N, 2D]
    Xp = X[inv]                                          # row n = new-id n
    htab = Xp[:, :D].copy()

    # per-chunk table row id (relative to that chunk's Shared tensor)
    nid = np.arange(N)
    c_of = nid // NS
    w_of = (nid % NS) // 128
    s_of = (nid % NS) % 128
    wchunk = np.searchsorted(cfg.wstart[1:], w_of, side="right")  # [N] 0..NCC-1
    rows_cc = np.array(cfg.rows_cc)
    rowbase_w = np.zeros(W, np.int64)       # row offset of window w in chunk
    for w in range(W):
        k = int(np.searchsorted(cfg.wstart[1:], w, side="right"))
        rowbase_w[w] = sum(128 if w2 < W - 1 else cfg.lastw
                           for w2 in range(cfg.wstart[k], w))
    rid_all = c_of * rows_cc[wchunk] + rowbase_w[w_of] + s_of   # within chunk

    src_n = new_id[np.asarray(src)]
    dst_n = new_id[np.asarray(dst)]
    cnt = np.bincount(dst_n, minlength=N).astype(np.float64)
    inv_cnt = (1.0 / np.maximum(cnt, 1.0)).astype(np.float32)
    kidx = wchunk[src_n]                                 # src table chunk
    rsrc = rid_all[src_n]                                # row within chunk
    core = dst_n // NS
    win = (dst_n % NS) // 128
    slot = (dst_n % NS) % 128
    key = (core * W + win) * NCC + kidx
    o = np.argsort(key, kind="stable")
    rsrc_s, slot_s = rsrc[o], slot[o]
    invc_s = inv_cnt[dst_n[o]]
    bounds = np.searchsorted(key[o], np.arange(C * W * NCC + 1))
    sizes = np.diff(bounds).reshape(C, W, NCC)
    kc = np.maximum(1, (sizes.max(axis=0) + 127) // 128).T   # [NCC, W]
    cfg.kc = kc.astype(int).tolist()
    totcols = int(kc.sum())
    nchmax = int(kc.sum(axis=0).max())
    iota_t = np.broadcast_to(
        np.tile(np.arange(128, dtype=BF16), nchmax), (128, nchmax * 128)).copy()

    in_maps = []
    for c in range(C):
        drel = np.full((128, totcols), 300.0, np.float32)
        invc = np.zeros((128, totcols), np.float32)
        wixs = []                 # wrapped gather idx pieces, (g, k) order
        cbase = 0                 # column base of current group
        for g in range((W + G - 1) // G):
            wins = list(range(g * G, min((g + 1) * G, W)))
            for k in range(NCC):
                seq = []
                for w in wins:
                    a, b = (bounds[(c * W + w) * NCC + k],
                            bounds[(c * W + w) * NCC + k + 1])
                    n = b - a
                    kw = int(kc[k][w])
                    buf_i = np.zeros(kw * 128, np.int64)
                    buf_d = np.full(kw * 128, 300.0, np.float32)
                    buf_v = np.zeros(kw * 128, np.float32)
                    buf_i[:n] = rsrc_s[a:b]
                    buf_d[:n] = slot_s[a:b]
                    buf_v[:n] = invc_s[a:b]
                    seq.append(buf_i)
                    cb = (cbase
                          + sum(int(kc[k2][w2]) for w2 in wins for k2 in range(NCC)
                                if (w2 < w))
                          + sum(int(kc[k2][w]) for k2 in range(k)))
                    drel[:, cb:cb + kw] = buf_d.reshape(kw, 128).T
                    invc[:, cb:cb + kw] = buf_v.reshape(kw, 128).T
                wixs.append(_wrap_idx(np.concatenate(seq)))
            cbase += sum(int(kc[k2][w2]) for w2 in wins for k2 in range(NCC))
        wix = np.concatenate(wixs, axis=1)

        xT = np.zeros((2 * D, cfg.NSP), BF16)
        xT[:, :NS] = Xp[c * NS:(c + 1) * NS].T
        in_maps.append(dict(
            xT=xT,
            hrows=htab[c * NS:(c + 1) * NS].copy(),
            wix=wix,
            iota_t=iota_t,
            drel=drel,
            invc=invc,
            Wm=Wm.astype(BF16),
            W_self=W_self.astype(BF16),
            W_neigh=W_neigh.astype(BF16),
            Wg=Wg.astype(BF16),
            bm=bm.reshape(1, D).astype(BF16),
            bs_col=b_sage.reshape(D, 1).astype(np.float32),
            bg_col=bg.reshape(D, 1).astype(np.float32),
            ln_g=np.broadcast_to(ln_g.astype(np.float32), (128, D)).copy(),
            ln_b=np.broadcast_to(ln_b.astype(np.float32), (128, D)).copy(),
        ))
    return in_maps, new_id


# --------------------------------------------------------------- device build

def build(cfg):
    N, C, NS, W, D, TW = cfg.N, cfg.C, cfg.NS, cfg.W, cfg.D, cfg.TW
    G, NCC = cfg.G, cfg.NCC
    kc = cfg.kc
    totcols = sum(sum(r) for r in kc)
    wixcols = totcols * 8                    # 128*totcols/16
    nc = bacc.Bacc("TRN2", target_bir_lowering=False, debug=False,
                   enable_asserts=False, num_devices=C,
                   num_swdge_queues=4)

    xT = nc.dram_tensor("xT", [2 * D, cfg.NSP], BF, kind="ExternalInput")
    hrows = nc.dram_tensor("hrows", [NS, D], BF, kind="ExternalInput")
    wix_d = nc.dram_tensor("wix", [128, wixcols], I16, kind="ExternalInput")
    drel = nc.dram_tensor("drel", [128, totcols], FP, kind="ExternalInput")
    invc = nc.dram_tensor("invc", [128, totcols], FP, kind="ExternalInput")
    Wm = nc.dram_tensor("Wm", [2 * D, D], BF, kind="ExternalInput")
    W_self = nc.dram_tensor("W_self", [D, D], BF, kind="ExternalInput")
    W_neigh = nc.dram_tensor("W_neigh", [D, D], BF, kind="ExternalInput")
    Wg = nc.dram_tensor("Wg", [2 * D, D], BF, kind="ExternalInput")
    bm = nc.dram_tensor("bm", [1, D], BF, kind="ExternalInput")
    bs_col = nc.dram_tensor("bs_col", [D, 1], FP, kind="ExternalInput")
    bg_col = nc.dram_tensor("bg_col", [D, 1], FP, kind="ExternalInput")
    ln_g = nc.dram_tensor("ln_g", [128, D], FP, kind="ExternalInput")
    ln_b = nc.dram_tensor("ln_b", [128, D], FP, kind="ExternalInput")
    nchmax = max(sum(kc[k][w] for k in range(NCC)) for w in range(W))
    iota_t = nc.dram_tensor("iota_t", [128, nchmax * 128], BF,
                            kind="ExternalInput")
    out = nc.dram_tensor("out", [D, cfg.NSP], FP, kind="ExternalOutput")

    with tile.TileContext(nc) as tc:
        with (
            tc.tile_pool(name="const", bufs=1) as cp,
            tc.tile_pool(name="p1", bufs=8) as p1,
            tc.tile_pool(name="sb", bufs=4) as sb,
            tc.tile_pool(name="vtp", bufs=3) as vp,
            tc.tile_pool(name="p3", bufs=2) as p3,
            tc.tile_pool(name="ps", bufs=2, space="PSUM") as ps,
            tc.tile_pool(name="ps1", bufs=2, space="PSUM") as ps1,
            tc.tile_pool(name="dram", bufs=1, space="DRAM") as dr,
        ):
            # ---- resident constants / tables
            iotaB = cp.tile([128, nchmax * 128], BF)
            nc.sync.dma_start(iotaB[:], iota_t[:])
            ones1 = cp.tile([1, 128], BF)
            nc.vector.memset(ones1[:], 1.0)
            wm_sb0 = cp.tile([D, D], BF)
            nc.sync.dma_start(wm_sb0[:], Wm[0:D, :])
            wm_sb1 = cp.tile([D, D], BF)
            nc.sync.dma_start(wm_sb1[:], Wm[D:2 * D, :])
            ws_sb = cp.tile([D, D], BF)
            nc.sync.dma_start(ws_sb[:], W_self[:])
            wn_sb = cp.tile([D, D], BF)
            nc.sync.dma_start(wn_sb[:], W_neigh[:])
            wg_sb0 = cp.tile([D, D], BF)
            nc.sync.dma_start(wg_sb0[:], Wg[0:D, :])
            wg_sb1 = cp.tile([D, D], BF)
            nc.sync.dma_start(wg_sb1[:], Wg[D:2 * D, :])
            bm_sb = cp.tile([1, D], BF)
            nc.sync.dma_start(bm_sb[:], bm[:])
            bs_sb = cp.tile([D, 1], FP)
            nc.sync.dma_start(bs_sb[:], bs_col[:])
            bg_sb = cp.tile([D, 1], FP)
            nc.sync.dma_start(bg_sb[:], bg_col[:])
            lng_sb = cp.tile([128, D], FP)
            nc.sync.dma_start(lng_sb[:], ln_g[:])
            lnb_sb = cp.tile([128, D], FP)
            nc.sync.dma_start(lnb_sb[:], ln_b[:])
            wix = cp.tile([128, wixcols], I16)
            nc.sync.dma_start(wix[:], wix_d[:])
            dre_all = cp.tile([128, totcols], FP)
            nc.sync.dma_start(dre_all[:], drel[:])
            ivc_all = cp.tile([128, totcols], FP)
            nc.sync.dma_start(ivc_all[:], invc[:])
            x0_all = cp.tile([128, cfg.NSP], BF)
            nc.sync.dma_start(x0_all[:], xT[0:128, :])
            x1_all = cp.tile([128, cfg.NSP], BF)
            nc.sync.dma_start(x1_all[:], xT[128:256, :])

            cc_ins = [dr.tile([cfg.rows_cc[k], TW], BF, name=f"cc_in{k}")
                      for k in range(NCC)]
            cc_outs = [dr.tile([C * cfg.rows_cc[k], TW], BF,
                               addr_space="Shared", name=f"cc_out{k}")
                       for k in range(NCC)]

            # ---- phase 1 (chunked): per-node msg = relu(LN(X @ Wm + bm));
            #      each chunk's table rows all-gather as soon as they're done
            for k in range(NCC):
                wins = list(range(cfg.wstart[k], cfg.wstart[k + 1]))
                rbase = cfg.wstart[k] * 128
                nc.sync.dma_start(
                    cc_ins[k][0:cfg.rows_cc[k], 0:D],
                    hrows[rbase:rbase + cfg.rows_cc[k], :])
                for t in wins:
                    rows = 128 if t < W - 1 else cfg.lastw
                    tc0, tc1 = t * 128, (t + 1) * 128
                    rloc = (t - cfg.wstart[k]) * 128
                    pe = ps1.tile([128, D], FP, tag="pe")
                    nc.tensor.matmul(out=pe[:], lhsT=x0_all[:, tc0:tc1],
                                     rhs=wm_sb0[:], start=True, stop=False)
                    nc.tensor.matmul(out=pe[:], lhsT=x1_all[:, tc0:tc1],
                                     rhs=wm_sb1[:], start=False, stop=False)
                    nc.tensor.matmul(out=pe[:], lhsT=ones1[:], rhs=bm_sb[:],
                                     start=False, stop=True)
                    st6 = p1.tile([128, 6], FP, tag="st6")
                    nc.vector.bn_stats(st6[:], pe[:])
                    mv = p1.tile([128, 2], FP, tag="mv")
                    nc.vector.bn_aggr(mv[:], st6[:])
                    rinv = p1.tile([128, 1], FP, tag="rinv")
                    nc.vector.tensor_scalar_add(rinv[:], mv[:, 1:2], EPS)
                    nc.vector.reciprocal(rinv[:], rinv[:])
                    s = p1.tile([128, 1], FP, tag="s")
                    nc.scalar.sqrt(s[:], rinv[:])
                    nmean = p1.tile([128, 1], FP, tag="nmean")
                    nc.vector.tensor_scalar(out=nmean[:], in0=mv[:, 0:1],
                                            scalar1=s[:, :1], scalar2=-1.0,
                                            op0=mybir.AluOpType.mult,
                                            op1=mybir.AluOpType.mult)
                    u = p1.tile([128, D], FP, tag="u")
                    nc.scalar.activation(u[:], pe[:],
                                         mybir.ActivationFunctionType.Identity,
                                         bias=nmean[:, :1], scale=s[:, :1])
                    nc.vector.tensor_tensor(out=u[:], in0=u[:], in1=lng_sb[:],
                                            op=mybir.AluOpType.mult)
                    nc.vector.tensor_tensor(out=u[:], in0=u[:], in1=lnb_sb[:],
                                            op=mybir.AluOpType.add)
                    msg = p1.tile([128, D], BF, tag="msg")
                    nc.scalar.activation(msg[:], u[:],
                                         mybir.ActivationFunctionType.Relu)
                    nc.sync.dma_start(cc_ins[k][rloc:rloc + rows, D:TW],
                                      msg[:rows, :])
                nc.gpsimd.collective_compute(
                    "AllGather", mybir.AluOpType.bypass,
                    replica_groups=[list(range(C))],
                    ins=[cc_ins[k][:, :].opt()],
                    outs=[cc_outs[k][:, :].opt()],
                )

            # ---- phase 2+3: bulk edge gathers + transposed segment-sums +
            #      gated fusion per destination window
            wo = 0
            colb = 0
            qn = 0
            for g in range((W + G - 1) // G):
                wins = list(range(g * G, min((g + 1) * G, W)))
                vts = []
                for k in range(NCC):
                    Kgk = 128 * sum(kc[k][w] for w in wins)
                    vt = vp.tile([128, (Kgk // 128) * TW], BF, tag=f"vt{k}",
                                 name=f"vt{k}_{g}")
                    pos = 0
                    while pos < Kgk:
                        kn = min(KMAX, Kgk - pos)
                        nc.gpsimd.dma_gather(
                            out_ap=vt[:, (pos // 128) * TW:
                                      ((pos + kn) // 128) * TW].rearrange(
                                          "p (c e) -> p c e", e=TW),
                            in_ap=cc_outs[k][:, :],
                            idxs_ap=wix[:, wo + pos // 16:
                                        wo + (pos + kn) // 16],
                            num_idxs=kn, num_idxs_reg=kn, elem_size=TW,
                            queue_num=qn % 4)
                        qn += 1
                        pos += kn
                    wo += Kgk // 16
                    vts.append(vt)
                soff = [0] * NCC
                for w in wins:
                    rows = 128 if w < W - 1 else cfg.lastw
                    nmT_t = ps.tile([128, 128], FP, tag="nmT")
                    caT_t = ps.tile([128, 128], FP, tag="caT")
                    nmT = nmT_t[:, :]
                    caT = caT_t[:, :]
                    nch = sum(kc[k][w] for k in range(NCC))
                    A_all = sb.tile([128, nchmax * 128], BF, tag="A")
                    nc.vector.tensor_tensor(
                        out=A_all[:, 0:nch * 128].rearrange(
                            "p (c e) -> p c e", e=128),
                        in0=iotaB[:, 0:nch * 128].rearrange(
                            "p (c e) -> p c e", e=128),
                        in1=dre_all[:, colb:colb + nch].unsqueeze(2)
                            .to_broadcast([128, nch, 128]),
                        op=mybir.AluOpType.is_equal)
                    nc.vector.tensor_tensor(
                        out=A_all[:, 0:nch * 128].rearrange(
                            "p (c e) -> p c e", e=128),
                        in0=A_all[:, 0:nch * 128].rearrange(
                            "p (c e) -> p c e", e=128),
                        in1=ivc_all[:, colb:colb + nch].unsqueeze(2)
                            .to_broadcast([128, nch, 128]),
                        op=mybir.AluOpType.mult)
                    ci = 0
                    for k in range(NCC):
                        for j in range(kc[k][w]):
                            off = (soff[k] + j) * TW
                            nc.tensor.matmul(
                                out=nmT, lhsT=vts[k][:, off:off + D],
                                rhs=A_all[:, ci * 128:(ci + 1) * 128],
                                start=(ci == 0), stop=(ci == nch - 1))
                            nc.tensor.matmul(
                                out=caT, lhsT=vts[k][:, off + D:off + TW],
                                rhs=A_all[:, ci * 128:(ci + 1) * 128],
                                start=(ci == 0), stop=(ci == nch - 1))
                            ci += 1
                    for k in range(NCC):
                        soff[k] += kc[k][w]
                    colb += nch
                    # gated fusion in transposed (channel x node) layout
                    nmT_bf = p3.tile([128, 128], BF, tag="nmT_bf")
                    nc.scalar.activation(nmT_bf[:], nmT,
                                         mybir.ActivationFunctionType.Copy)
                    caT_bf = p3.tile([128, 128], BF, tag="caT_bf")
                    nc.scalar.activation(caT_bf[:], caT,
                                         mybir.ActivationFunctionType.Copy)
                    pstdT = ps.tile([128, 128], FP, tag="mm")
                    nc.tensor.matmul(out=pstdT[:], lhsT=ws_sb[:],
                                     rhs=x0_all[:, w * 128:(w + 1) * 128],
                                     start=True, stop=False)
                    nc.tensor.matmul(out=pstdT[:], lhsT=wn_sb[:],
                                     rhs=nmT_bf[:], start=False, stop=True)
                    stdT_bf = p3.tile([128, 128], BF, tag="stdT_bf")
                    nc.scalar.activation(stdT_bf[:], pstdT[:],
                                         mybir.ActivationFunctionType.Identity,
                                         bias=bs_sb[:, :1])
                    pg = ps.tile([128, 128], FP, tag="mm")
                    nc.tensor.matmul(out=pg[:], lhsT=wg_sb0[:],
                                     rhs=stdT_bf[:], start=True, stop=False)
                    nc.tensor.matmul(out=pg[:], lhsT=wg_sb1[:],
                                     rhs=caT_bf[:], start=False, stop=True)
                    gt = p3.tile([128, 128], FP, tag="gt")
                    nc.scalar.activation(gt[:], pg[:],
                                         mybir.ActivationFunctionType.Sigmoid,
                                         bias=bg_sb[:, :1])
                    d = p3.tile([128, 128], FP, tag="d")
                    nc.vector.tensor_tensor(out=d[:], in0=stdT_bf[:],
                                            in1=caT_bf[:],
                                            op=mybir.AluOpType.subtract)
                    o = p3.tile([128, 128], FP, tag="o")
                    nc.vector.tensor_tensor(out=o[:], in0=gt[:], in1=d[:],
                                            op=mybir.AluOpType.mult)
                    nc.vector.tensor_tensor(out=o[:], in0=o[:], in1=caT_bf[:],
                                            op=mybir.AluOpType.add)
                    nc.sync.dma_start(out[:, w * 128:w * 128 + rows],
                                      o[:, :rows])

    nc.compile()
    return nc


# ----------------------------------------------------------------- entrypoint

def _install_ntff_shim():
    """Registers antenv.axon_hooks so trace=True can capture neuron profiles
    under axon (the agent image lacks the module)."""
    import contextlib, ctypes, types
    if "antenv.axon_hooks" in sys.modules:
        return
    try:
        lib = ctypes.CDLL("/opt/axon/libaxon_pjrt.so")
        assert hasattr(lib, "axon_start_nrt_profile")
    except Exception:
        return
    lib.axon_start_nrt_profile.argtypes = [ctypes.POINTER(ctypes.c_int64), ctypes.c_size_t]
    lib.axon_start_nrt_profile.restype = ctypes.c_int64
    lib.axon_stop_nrt_profile.argtypes = [ctypes.c_char_p]
    lib.axon_stop_nrt_profile.restype = ctypes.c_int64

    @contextlib.contextmanager
    def _hook(output_dir, device_ids):
        import jax
        jax.devices()
        if device_ids:
            ids = (ctypes.c_int64 * len(device_ids))(*device_ids)
            rc = lib.axon_start_nrt_profile(ids, len(device_ids))
        else:
            rc = lib.axon_start_nrt_profile(None, 0)
        if rc != 0:
            raise RuntimeError(f"axon_start_nrt_profile rc={rc}")
        try:
            yield
        finally:
            lib.axon_stop_nrt_profile(str(output_dir).encode())

    mod = types.ModuleType("antenv.axon_hooks")
    mod.get_axon_ntff_profile_hook = lambda: _hook
    mod.set_axon_ntff_profile_hook = lambda h: None
    sys.modules["antenv.axon_hooks"] = mod


def run(inputs, cfg, trace=False):
    if trace:
        _install_ntff_shim()
    in_maps, new_id = prep_inputs(**inputs, cfg=cfg)
    nc = build(cfg)
    res = bass_utils.run_bass_kernel_spmd(
        nc, in_maps, core_ids=list(range(cfg.C)), trace=trace)
    outp = np.concatenate(
        [res.results[c]["out"][:, :cfg.NS].T for c in range(cfg.C)], axis=0)
    result = outp[new_id].astype(np.float32)
    return result, res


def kernel(**inputs) -> np.ndarray:
    h = np.asarray(inputs["h"])
    cfg = Cfg(N=h.shape[0], E=np.asarray(inputs["src"]).shape[0])
    inputs = {k: np.asarray(v) for k, v in inputs.items()}
    result, _ = run(inputs, cfg)
    return result


if __name__ == "__main__":
    pass


# revision 16
# speedup vs baseline: 1.3987x; 1.3987x over previous
"""ConversationAwareSAGEConv on 8 Trainium2 NeuronCores (Bass/Tile).

Algorithm notes:
- The per-edge MLP message e = concat(h,ctx)[src] @ Wm + bm depends only on
  the src node, so LN(e)+relu is computed per NODE (N rows) instead of per
  edge (E rows): 10x less work.
- Nodes are relabeled by a degree-balanced permutation so every 128-node
  destination window receives a near-equal number of edges.
- Sharding: nodes are split into 8 contiguous shards. Each core computes msg
  for its shard (node-parallel matmul + LN). The [h|msg] table is all-gathered
  in NCC chunks, each into its own Shared buffer, pipelining the collective
  behind phase 1. Edges are classified by the src node's table chunk, which
  also keeps gather indices < 13.5k (int16-safe).
- Edge rows are fetched with gpsimd.dma_gather (InstDMAGatherAnt): ONE SWDGE
  instruction gathers up to KMAX rows (row i -> partition i%128, slot i//128),
  amortizing the ~1us per-instruction descriptor-generation cost that
  dominates indirect_dma_start (the hardware caps one instruction at ~1-2k
  descriptors, hence KMAX).
- Segment sums run transposed: matmul(lhsT=rows, rhs=onehot) accumulates
  nmT/caT = (features x dst-slot) directly in PSUM; 1/cnt is baked into the
  one-hot host-side; SAGE/gate biases fold into scalar-engine activations;
  the output is written transposed and the host untransposes.
"""
import sys

sys.path.insert(0, "/opt/trn_rl_repo")

import numpy as np
import ml_dtypes

import concourse.bass as bass
import concourse.bacc as bacc
import concourse.tile as tile
from concourse import mybir
import concourse.bass_utils as bass_utils

BF16 = ml_dtypes.bfloat16
EPS = 1e-5
FP = mybir.dt.float32
BF = mybir.dt.bfloat16
I16 = mybir.dt.int16
KMAX = 1024                                   # rows per dma_gather instruction


class Cfg:
    def __init__(self, N, E, C=8, D=128):
        assert N % C == 0
        self.N, self.E, self.C, self.D = N, E, C, D
        self.NS = N // C                      # nodes per shard
        self.W = (self.NS + 127) // 128       # windows per core
        self.NSP = self.W * 128
        self.lastw = self.NS - (self.W - 1) * 128
        self.TW = 256                         # table row: h|msg
        self.G = 7                            # windows per gather group
        self.NCC = 2                          # src index ranges (int16 limit)
        self.SPLIT = (self.N + 1) // 2        # src range boundary
        # filled by prep_inputs: kc[k][w] = gather chunks for (window, range)
        self.kc = None


# ----------------------------------------------------------------- host prep

def _balanced_perm(dst, cfg):
    """new_id[old] such that every 128-node window gets ~equal in-edges."""
    import heapq
    N, C, W, NS = cfg.N, cfg.C, cfg.W, cfg.NS
    deg = np.bincount(dst, minlength=N)
    order = np.argsort(-deg, kind="stable")
    caps = np.full(C * W, 128, np.int64)
    caps[W - 1::W] = cfg.lastw
    heap = [(0, int(w)) for w in range(C * W)]
    heapq.heapify(heap)
    fill = np.zeros(C * W, np.int64)
    new_id = np.empty(N, np.int64)
    for old in order:
        d = int(deg[old])
        while True:
            load, w = heapq.heappop(heap)
            if fill[w] < caps[w]:
                break
        new_id[old] = (w // W) * NS + (w % W) * 128 + fill[w]
        fill[w] += 1
        if fill[w] < caps[w]:
            heapq.heappush(heap, (load + d, w))
    return new_id


def _wrap_idx(seq):
    """[K] -> [128, K//16] int16 in the swdge wrapped+replicated layout."""
    blk = seq.reshape(-1, 16).T.astype(np.int16)      # [16, K//16]
    return np.tile(blk, (8, 1))


def prep_inputs(h, ctx, src, dst, W_self, W_neigh, b_sage, Wm, bm, ln_g, ln_b,
                Wg, bg, cfg):
    N, C, NS, W, D, G = cfg.N, cfg.C, cfg.NS, cfg.W, cfg.D, cfg.G
    NCC = cfg.NCC
    new_id = _balanced_perm(np.asarray(dst), cfg)
    inv = np.empty(N, np.int64)
    inv[new_id] = np.arange(N)

    X = np.concatenate([h, ctx], axis=1).astype(BF16)   # [N, 2D]
    Xp = X[inv]                                          # row n = new-id n
    htab = Xp[:, :D].copy()

    src_n = new_id[np.asarray(src)]
    dst_n = new_id[np.asarray(dst)]
    cnt = np.bincount(dst_n, minlength=N).astype(np.float64)
    inv_cnt = (1.0 / np.maximum(cnt, 1.0)).astype(np.float32)
    kidx = src_n // cfg.SPLIT                            # src index range
    rsrc = src_n - kidx * cfg.SPLIT                      # row within range
    core = dst_n // NS
    win = (dst_n % NS) // 128
    slot = (dst_n % NS) % 128
    key = (core * W + win) * NCC + kidx
    o = np.argsort(key, kind="stable")
    rsrc_s, slot_s = rsrc[o], slot[o]
    invc_s = inv_cnt[dst_n[o]]
    bounds = np.searchsorted(key[o], np.arange(C * W * NCC + 1))
    sizes = np.diff(bounds).reshape(C, W, NCC)
    kc = np.maximum(1, (sizes.max(axis=0) + 127) // 128).T   # [NCC, W]
    cfg.kc = kc.astype(int).tolist()
    totcols = int(kc.sum())
    nchmax = int(kc.sum(axis=0).max())
    iota_t = np.broadcast_to(
        np.tile(np.arange(128, dtype=BF16), nchmax), (128, nchmax * 128)).copy()

    in_maps = []
    for c in range(C):
        drel = np.full((128, totcols), 300.0, np.float32)
        invc = np.zeros((128, totcols), np.float32)
        wixs = []                 # wrapped gather idx pieces, (g, k) order
        cbase = 0                 # column base of current group
        for g in range((W + G - 1) // G):
            wins = list(range(g * G, min((g + 1) * G, W)))
            for k in range(NCC):
                seq = []
                for w in wins:
                    a, b = (bounds[(c * W + w) * NCC + k],
                            bounds[(c * W + w) * NCC + k + 1])
                    n = b - a
                    kw = int(kc[k][w])
                    buf_i = np.zeros(kw * 128, np.int64)
                    buf_d = np.full(kw * 128, 300.0, np.float32)
                    buf_v = np.zeros(kw * 128, np.float32)
                    buf_i[:n] = rsrc_s[a:b]
                    buf_d[:n] = slot_s[a:b]
                    buf_v[:n] = invc_s[a:b]
                    seq.append(buf_i)
                    cb = (cbase
                          + sum(int(kc[k2][w2]) for w2 in wins for k2 in range(NCC)
                                if (w2 < w))
                          + sum(int(kc[k2][w]) for k2 in range(k)))
                    drel[:, cb:cb + kw] = buf_d.reshape(kw, 128).T
                    invc[:, cb:cb + kw] = buf_v.reshape(kw, 128).T
                wixs.append(_wrap_idx(np.concatenate(seq)))
            cbase += sum(int(kc[k2][w2]) for w2 in wins for k2 in range(NCC))
        wix = np.concatenate(wixs, axis=1)

        xT = np.zeros((2 * D, cfg.NSP), BF16)
        xT[:, :NS] = Xp[c * NS:(c + 1) * NS].T
        in_maps.append(dict(
            xT=xT,
            hrows=htab[c * NS:(c + 1) * NS].copy(),
            wix=wix,
            iota_t=iota_t,
            drel=drel,
            invc=invc,
            Wm=Wm.astype(BF16),
            W_self=W_self.astype(BF16),
            W_neigh=W_neigh.astype(BF16),
            Wg=Wg.astype(BF16),
            bm=bm.reshape(1, D).astype(BF16),
            bs_col=b_sage.reshape(D, 1).astype(np.float32),
            bg_col=bg.reshape(D, 1).astype(np.float32),
            ln_g=np.broadcast_to(ln_g.astype(np.float32), (128, D)).copy(),
            ln_b=np.broadcast_to(ln_b.astype(np.float32), (128, D)).copy(),
        ))
    return in_maps, new_id


# --------------------------------------------------------------- device build

def build(cfg):
    N, C, NS, W, D, TW = cfg.N, cfg.C, cfg.NS, cfg.W, cfg.D, cfg.TW
    G, NCC = cfg.G, cfg.NCC
    kc = cfg.kc
    totcols = sum(sum(r) for r in kc)
    wixcols = totcols * 8                    # 128*totcols/16
    nc = bacc.Bacc("TRN2", target_bir_lowering=False, debug=False,
                   enable_asserts=False, num_devices=C,
                   num_swdge_queues=4)

    xT = nc.dram_tensor("xT", [2 * D, cfg.NSP], BF, kind="ExternalInput")
    hrows = nc.dram_tensor("hrows", [NS, D], BF, kind="ExternalInput")
    wix_d = nc.dram_tensor("wix", [128, wixcols], I16, kind="ExternalInput")
    drel = nc.dram_tensor("drel", [128, totcols], FP, kind="ExternalInput")
    invc = nc.dram_tensor("invc", [128, totcols], FP, kind="ExternalInput")
    Wm = nc.dram_tensor("Wm", [2 * D, D], BF, kind="ExternalInput")
    W_self = nc.dram_tensor("W_self", [D, D], BF, kind="ExternalInput")
    W_neigh = nc.dram_tensor("W_neigh", [D, D], BF, kind="ExternalInput")
    Wg = nc.dram_tensor("Wg", [2 * D, D], BF, kind="ExternalInput")
    bm = nc.dram_tensor("bm", [1, D], BF, kind="ExternalInput")
    bs_col = nc.dram_tensor("bs_col", [D, 1], FP, kind="ExternalInput")
    bg_col = nc.dram_tensor("bg_col", [D, 1], FP, kind="ExternalInput")
    ln_g = nc.dram_tensor("ln_g", [128, D], FP, kind="ExternalInput")
    ln_b = nc.dram_tensor("ln_b", [128, D], FP, kind="ExternalInput")
    nchmax = max(sum(kc[k][w] for k in range(NCC)) for w in range(W))
    iota_t = nc.dram_tensor("iota_t", [128, nchmax * 128], BF,
                            kind="ExternalInput")
    out = nc.dram_tensor("out", [D, cfg.NSP], FP, kind="ExternalOutput")

    with tile.TileContext(nc) as tc:
        with (
            tc.tile_pool(name="const", bufs=1) as cp,
            tc.tile_pool(name="p1", bufs=8) as p1,
            tc.tile_pool(name="sb", bufs=4) as sb,
            tc.tile_pool(name="vtp", bufs=2) as vp,
            tc.tile_pool(name="p3", bufs=3) as p3,
            tc.tile_pool(name="ps", bufs=2, space="PSUM") as ps,
            tc.tile_pool(name="ps1", bufs=2, space="PSUM") as ps1,
            tc.tile_pool(name="dram", bufs=1, space="DRAM") as dr,
        ):
            # ---- resident constants / tables
            iotaB = cp.tile([128, nchmax * 128], BF)
            nc.sync.dma_start(iotaB[:], iota_t[:])
            ones1 = cp.tile([1, 128], BF)
            nc.vector.memset(ones1[:], 1.0)
            wm_sb0 = cp.tile([D, D], BF)
            nc.sync.dma_start(wm_sb0[:], Wm[0:D, :])
            wm_sb1 = cp.tile([D, D], BF)
            nc.sync.dma_start(wm_sb1[:], Wm[D:2 * D, :])
            ws_sb = cp.tile([D, D], BF)
            nc.sync.dma_start(ws_sb[:], W_self[:])
            wn_sb = cp.tile([D, D], BF)
            nc.sync.dma_start(wn_sb[:], W_neigh[:])
            wg_sb0 = cp.tile([D, D], BF)
            nc.sync.dma_start(wg_sb0[:], Wg[0:D, :])
            wg_sb1 = cp.tile([D, D], BF)
            nc.sync.dma_start(wg_sb1[:], Wg[D:2 * D, :])
            bm_sb = cp.tile([1, D], BF)
            nc.sync.dma_start(bm_sb[:], bm[:])
            bs_sb = cp.tile([D, 1], FP)
            nc.sync.dma_start(bs_sb[:], bs_col[:])
            bg_sb = cp.tile([D, 1], FP)
            nc.sync.dma_start(bg_sb[:], bg_col[:])
            lng_sb = cp.tile([128, D], FP)
            nc.sync.dma_start(lng_sb[:], ln_g[:])
            lnb_sb = cp.tile([128, D], FP)
            nc.sync.dma_start(lnb_sb[:], ln_b[:])
            wix = cp.tile([128, wixcols], I16)
            nc.sync.dma_start(wix[:], wix_d[:])
            dre_all = cp.tile([128, totcols], FP)
            nc.sync.dma_start(dre_all[:], drel[:])
            ivc_all = cp.tile([128, totcols], FP)
            nc.sync.dma_start(ivc_all[:], invc[:])
            x0_all = cp.tile([128, cfg.NSP], BF)
            nc.sync.dma_start(x0_all[:], xT[0:128, :])
            x1_all = cp.tile([128, cfg.NSP], BF)
            nc.sync.dma_start(x1_all[:], xT[128:256, :])

            cc_in = dr.tile([NS, TW], BF)
            cc_out = dr.tile([N, TW], BF, addr_space="Shared")
            nc.sync.dma_start(cc_in[0:NS, 0:D], hrows[:, :])

            # ---- phase 1: per-node msg = relu(LN(X @ Wm + bm)); LN scale
            #      factors are computed for ALL windows in one batched pass
            mv_all = cp.tile([128, W, 2], FP)
            pe_bf = cp.tile([128, W * 128], BF)
            for t in range(W):
                tc0, tc1 = t * 128, (t + 1) * 128
                pe = ps1.tile([128, D], FP, tag="pe")
                nc.tensor.matmul(out=pe[:], lhsT=x0_all[:, tc0:tc1],
                                 rhs=wm_sb0[:], start=True, stop=False)
                nc.tensor.matmul(out=pe[:], lhsT=x1_all[:, tc0:tc1],
                                 rhs=wm_sb1[:], start=False, stop=False)
                nc.tensor.matmul(out=pe[:], lhsT=ones1[:], rhs=bm_sb[:],
                                 start=False, stop=True)
                st6 = p1.tile([128, 6], FP, tag="st6")
                nc.vector.bn_stats(st6[:], pe[:])
                nc.vector.bn_aggr(mv_all[:, t, :], st6[:])
                nc.scalar.activation(pe_bf[:, tc0:tc1], pe[:],
                                     mybir.ActivationFunctionType.Copy)
            s_all = cp.tile([128, W], FP)
            nc.vector.tensor_scalar_add(s_all[:], mv_all[:, :, 1], EPS)
            nc.vector.reciprocal(s_all[:], s_all[:])
            nc.scalar.sqrt(s_all[:], s_all[:])
            nmean_all = cp.tile([128, W], FP)
            nc.vector.tensor_tensor(out=nmean_all[:], in0=mv_all[:, :, 0],
                                    in1=s_all[:],
                                    op=mybir.AluOpType.mult)
            nc.vector.tensor_scalar_mul(nmean_all[:], nmean_all[:], -1.0)
            for t in range(W):
                rows = 128 if t < W - 1 else cfg.lastw
                tc0, tc1 = t * 128, (t + 1) * 128
                u = p1.tile([128, D], FP, tag="u")
                nc.scalar.activation(u[:], pe_bf[:, tc0:tc1],
                                     mybir.ActivationFunctionType.Identity,
                                     bias=nmean_all[:, t:t + 1],
                                     scale=s_all[:, t:t + 1])
                nc.vector.tensor_tensor(out=u[:], in0=u[:], in1=lng_sb[:],
                                        op=mybir.AluOpType.mult)
                nc.vector.tensor_tensor(out=u[:], in0=u[:], in1=lnb_sb[:],
                                        op=mybir.AluOpType.add)
                msg = p1.tile([128, D], BF, tag="msg")
                nc.scalar.activation(msg[:], u[:],
                                     mybir.ActivationFunctionType.Relu)
                nc.sync.dma_start(cc_in[t * 128:t * 128 + rows, D:TW],
                                  msg[:rows, :])
            nc.gpsimd.collective_compute(
                "AllGather", mybir.AluOpType.bypass,
                replica_groups=[list(range(C))],
                ins=[cc_in.opt()], outs=[cc_out.opt()],
            )

            # ---- phase 2+3: bulk edge gathers + transposed segment-sums +
            #      gated fusion per destination window
            wo = 0
            colb = 0
            qn = 0
            for g in range((W + G - 1) // G):
                wins = list(range(g * G, min((g + 1) * G, W)))
                vts = []
                for k in range(NCC):
                    Kgk = 128 * sum(kc[k][w] for w in wins)
                    vt = vp.tile([128, (Kgk // 128) * TW], BF, tag=f"vt{k}",
                                 name=f"vt{k}_{g}")
                    pos = 0
                    while pos < Kgk:
                        kn = min(KMAX, Kgk - pos)
                        nc.gpsimd.dma_gather(
                            out_ap=vt[:, (pos // 128) * TW:
                                      ((pos + kn) // 128) * TW].rearrange(
                                          "p (c e) -> p c e", e=TW),
                            in_ap=(cc_out[0:cfg.SPLIT, :] if k == 0
                                   else cc_out[cfg.SPLIT:N, :]),
                            idxs_ap=wix[:, wo + pos // 16:
                                        wo + (pos + kn) // 16],
                            num_idxs=kn, num_idxs_reg=kn, elem_size=TW,
                            queue_num=qn % 4)
                        qn += 1
                        pos += kn
                    wo += Kgk // 16
                    vts.append(vt)
                soff = [0] * NCC
                for w in wins:
                    rows = 128 if w < W - 1 else cfg.lastw
                    nmT_t = ps.tile([128, 128], FP, tag="nmT")
                    caT_t = ps.tile([128, 128], FP, tag="caT")
                    nmT = nmT_t[:, :]
                    caT = caT_t[:, :]
                    nch = sum(kc[k][w] for k in range(NCC))
                    A_all = sb.tile([128, nchmax * 128], BF, tag="A")
                    nc.vector.tensor_tensor(
                        out=A_all[:, 0:nch * 128].rearrange(
                            "p (c e) -> p c e", e=128),
                        in0=iotaB[:, 0:nch * 128].rearrange(
                            "p (c e) -> p c e", e=128),
                        in1=dre_all[:, colb:colb + nch].unsqueeze(2)
                            .to_broadcast([128, nch, 128]),
                        op=mybir.AluOpType.is_equal)
                    nc.vector.tensor_tensor(
                        out=A_all[:, 0:nch * 128].rearrange(
                            "p (c e) -> p c e", e=128),
                        in0=A_all[:, 0:nch * 128].rearrange(
                            "p (c e) -> p c e", e=128),
                        in1=ivc_all[:, colb:colb + nch].unsqueeze(2)
                            .to_broadcast([128, nch, 128]),
                        op=mybir.AluOpType.mult)
                    ci = 0
                    for k in range(NCC):
                        for j in range(kc[k][w]):
                            off = (soff[k] + j) * TW
                            nc.tensor.matmul(
                                out=nmT, lhsT=vts[k][:, off:off + D],
                                rhs=A_all[:, ci * 128:(ci + 1) * 128],
                                start=(ci == 0), stop=(ci == nch - 1))
                            nc.tensor.matmul(
                                out=caT, lhsT=vts[k][:, off + D:off + TW],
                                rhs=A_all[:, ci * 128:(ci + 1) * 128],
                                start=(ci == 0), stop=(ci == nch - 1))
                            ci += 1
                    for k in range(NCC):
                        soff[k] += kc[k][w]
                    colb += nch
                    # gated fusion in transposed (channel x node) layout
                    nmT_bf = p3.tile([128, 128], BF, tag="nmT_bf")
                    nc.scalar.activation(nmT_bf[:], nmT,
                                         mybir.ActivationFunctionType.Copy)
                    caT_bf = p3.tile([128, 128], BF, tag="caT_bf")
                    nc.scalar.activation(caT_bf[:], caT,
                                         mybir.ActivationFunctionType.Copy)
                    pstdT = ps.tile([128, 128], FP, tag="mm")
                    nc.tensor.matmul(out=pstdT[:], lhsT=ws_sb[:],
                                     rhs=x0_all[:, w * 128:(w + 1) * 128],
                                     start=True, stop=False)
                    nc.tensor.matmul(out=pstdT[:], lhsT=wn_sb[:],
                                     rhs=nmT_bf[:], start=False, stop=True)
                    stdT_bf = p3.tile([128, 128], BF, tag="stdT_bf")
                    nc.scalar.activation(stdT_bf[:], pstdT[:],
                                         mybir.ActivationFunctionType.Identity,
                                         bias=bs_sb[:, :1])
                    pg = ps.tile([128, 128], FP, tag="mm")
                    nc.tensor.matmul(out=pg[:], lhsT=wg_sb0[:],
                                     rhs=stdT_bf[:], start=True, stop=False)
                    nc.tensor.matmul(out=pg[:], lhsT=wg_sb1[:],
                                     rhs=caT_bf[:], start=False, stop=True)
                    gt = p3.tile([128, 128], FP, tag="gt")
                    nc.scalar.activation(gt[:], pg[:],
                                         mybir.ActivationFunctionType.Sigmoid,
                                         bias=bg_sb[:, :1])
                    d = p3.tile([128, 128], FP, tag="d")
                    nc.vector.tensor_tensor(out=d[:], in0=stdT_bf[:],
                                            in1=caT_bf[:],
                                            op=mybir.AluOpType.subtract)
                    o = p3.tile([128, 128], FP, tag="o")
                    nc.vector.tensor_tensor(out=o[:], in0=gt[:], in1=d[:],
                                            op=mybir.AluOpType.mult)
                    nc.vector.tensor_tensor(out=o[:], in0=o[:], in1=caT_bf[:],
                                            op=mybir.AluOpType.add)
                    nc.sync.dma_start(out[:, w * 128:w * 128 + rows],
                                      o[:, :rows])

    nc.compile()
    return nc


# ----------------------------------------------------------------- entrypoint

def _install_ntff_shim():
    """Registers antenv.axon_hooks so trace=True can capture neuron profiles
    under axon (the agent image lacks the module)."""
    import contextlib, ctypes, types
    if "antenv.axon_hooks" in sys.modules:
        return
    try:
        lib = ctypes.CDLL("/opt/axon/libaxon_pjrt.so")
        assert hasattr(lib, "axon_start_nrt_profile")
    except Exception:
        return
    lib.axon_start_nrt_profile.argtypes = [ctypes.POINTER(ctypes.c_int64), ctypes.c_size_t]
    lib.axon_start_nrt_profile.restype = ctypes.c_int64
    lib.axon_stop_nrt_profile.argtypes = [ctypes.c_char_p]
    lib.axon_stop_nrt_profile.restype = ctypes.c_int64

    @contextlib.contextmanager
    def _hook(output_dir, device_ids):
        import jax
        jax.devices()
        if device_ids:
            ids = (ctypes.c_int64 * len(device_ids))(*device_ids)
            rc = lib.axon_start_nrt_profile(ids, len(device_ids))
        else:
            rc = lib.axon_start_nrt_profile(None, 0)
        if rc != 0:
            raise RuntimeError(f"axon_start_nrt_profile rc={rc}")
        try:
            yield
        finally:
            lib.axon_stop_nrt_profile(str(output_dir).encode())

    mod = types.ModuleType("antenv.axon_hooks")
    mod.get_axon_ntff_profile_hook = lambda: _hook
    mod.set_axon_ntff_profile_hook = lambda h: None
    sys.modules["antenv.axon_hooks"] = mod


def run(inputs, cfg, trace=False):
    if trace:
        _install_ntff_shim()
    in_maps, new_id = prep_inputs(**inputs, cfg=cfg)
    nc = build(cfg)
    res = bass_utils.run_bass_kernel_spmd(
        nc, in_maps, core_ids=list(range(cfg.C)), trace=trace)
    outp = np.concatenate(
        [res.results[c]["out"][:, :cfg.NS].T for c in range(cfg.C)], axis=0)
    result = outp[new_id].astype(np.float32)
    return result, res


def kernel(**inputs) -> np.ndarray:
    h = np.asarray(inputs["h"])
    cfg = Cfg(N=h.shape[0], E=np.asarray(inputs["src"]).shape[0])
    inputs = {k: np.asarray(v) for k, v in inputs.items()}
    result, _ = run(inputs, cfg)
    return result


if __name__ == "__main__":
    pass
